# revision 1
# baseline (speedup 1.0000x reference)
"""Trainium2 Bass kernel: separable box filter (radius 4) on (8,3,1024,1024) fp32.

Equivalent to the reference:
    box(x) = diff(cumsum(diff(cumsum(x, H), H), W), W)    # truncated 9x9 box sum

Strategy (pure data parallel over the 24 (n,c) slices, 3 per core):
  - W pass entirely on DVE in ONE tensor_tensor_scan per tile:
        state[t] = state[t-1] + xpad[t] - xpad[t-9]
    over a zero-padded row buffer xpad = [0 x9 | x | 0 x4], which yields the
    truncated 9-tap running box sum S with S[w+4] = boxW(x)[w].
  - H pass on the PE: overlapping input tiles of 128 rows produce 120 output
    rows each via one banded weight matrix W[m, k] = 1 iff m <= k <= m+8
    (tile covers global rows 120t-4 .. 120t+123).
  - ACT copies PSUM -> SBUF, DMA out.
"""

import numpy as np

H = 1024
W = 1024
R = 4
D = 2 * R + 1  # 9-tap window
N_CORES = 8
SLICES_PER_CORE = 3  # 8*3 = 24 (n,c) slices / 8 cores
TILE_OUT = 120  # output rows per PE tile (128 input rows - 2*R)
N_TILES = 9  # ceil(1024 / 120); last tile emits 64 rows
P_W = D + W + R  # 9 left zeros + 1024 data + 4 right zeros
S_W = W + R  # scan output length (box sums ending at 0..1027)

_COMPILED = {}


def _band_weights():
    """lhsT for the H-pass band matmul: [K=128, M=120], lhsT[k, m] = 1 iff
    m <= k <= m+8 (out row m consumes in rows m..m+8 of the tile)."""
    k = np.arange(128)[:, None]
    m = np.arange(TILE_OUT)[None, :]
    return ((m <= k) & (k <= m + 2 * R)).astype(np.float32)


def _build():
    from concourse import bacc, mybir
    from concourse.tile import TileContext

    f32 = mybir.dt.float32
    nc = bacc.Bacc("TRN2", target_bir_lowering=False, debug=False,
                   num_devices=N_CORES)

    x = nc.dram_tensor("x", (SLICES_PER_CORE, H, W), f32,
                       kind="ExternalInput").ap()
    wp = nc.dram_tensor("wp", (128, TILE_OUT), f32, kind="ExternalInput").ap()
    out = nc.dram_tensor("out", (SLICES_PER_CORE, H, W), f32,
                         kind="ExternalOutput").ap()

    add = mybir.AluOpType.add
    sub = mybir.AluOpType.subtract
    act_copy = mybir.ActivationFunctionType.Copy

    from concourse.ap import AP

    xh = x.tensor
    oh = out.tensor

    def src_windows(s, t0, nt):
        # Overlapping 128-row windows: rows 120*t - 4 + p for t in
        # [t0, t0+nt), p in [0, 128).  Iteration order must match the
        # SBUF dest [p, t, w].
        off = s * H * W + (TILE_OUT * t0 - R) * W
        return AP(xh, off, [[W, 128], [TILE_OUT * W, nt], [1, W]])

    def dst_rows(s, t0, nt):
        # Output rows 120*t + p for t in [t0, t0+nt), p in [0, 120).
        off = s * H * W + TILE_OUT * t0 * W
        return AP(oh, off, [[W, TILE_OUT], [TILE_OUT * W, nt], [1, W]])

    # Tile-pair chunks: dependency granularity == DMA granularity, so each
    # scan waits only on its own ~1MB load and the pipeline is pair-granular
    # end to end (no slice barriers).
    CHUNKS = [(t, 1) for t in range(N_TILES)]

    with TileContext(nc) as tc:
        with tc.tile_pool(name="wts", bufs=1) as wpool, \
             tc.tile_pool(name="xp", bufs=1) as xpool, \
             tc.tile_pool(name="sc", bufs=8) as spool, \
             tc.tile_pool(name="outp", bufs=10) as opool, \
             tc.tile_pool(name="ps", bufs=8, space="PSUM") as pspool:
            wp_t = wpool.tile([128, TILE_OUT], f32)
            nc.sync.dma_start(wp_t[:], wp[:])

            # 9 persistent input buffers: chunk c uses buffer t (= c mod 9),
            # so buffer 0 always hosts t=0 tiles and buffer 8 always t=8.
            # All constant zero regions (row pads, out-of-image partition
            # ranges) are initialized ONCE here and never rewritten -- no
            # per-chunk memsets, no WAR edges in steady state.
            xbufs = []
            for t in range(N_TILES):
                xb = xpool.tile([128, P_W], f32, tag=f"xc{t}")
                nc.gpsimd.memset(xb[:, 0:D], 0.0)
                nc.gpsimd.memset(xb[:, D + W:P_W], 0.0)
                if t == 0:
                    nc.gpsimd.memset(xb[0:32, :], 0.0)
                if t == 8:
                    nc.gpsimd.memset(xb[64:128, :], 0.0)
                xbufs.append(xb)

            for s in range(SLICES_PER_CORE):
                for t in range(N_TILES):
                    xc = xbufs[t]
                    ineng = nc.sync
                    if t == 0:
                        ineng.dma_start(xc[4:128, D:D + W], x[s, 0:124, :])
                    elif t == 8:
                        ineng.dma_start(xc[0:68, D:D + W],
                                        x[s, 8 * TILE_OUT - R:H, :])
                    else:
                        ineng.dma_start(xc[:, D:D + W],
                                        src_windows(s, t, 1)[:, 0, :])

                    if t % 2 == 0:
                        oc = opool.tile([TILE_OUT, 2, W], f32, tag="oc")
                    oi = t % 2
                    m = min(TILE_OUT, H - TILE_OUT * t)  # output rows
                    # Running 9-tap box sum along W:
                    #   S[i] = S[i-1] + xpad[i] - xpad[i-9], i = 0..1027
                    # so S[w+4] = truncated boxW(x)[w].
                    st = spool.tile([128, S_W], f32)
                    nc.vector.tensor_tensor_scan(
                        st[:, :], xc[:, D:P_W], xc[:, 0:S_W], 0.0,
                        add, sub)
                    for hf in range(2):
                        w0 = 512 * hf
                        ps = pspool.tile([TILE_OUT, 512], f32)
                        nc.tensor.matmul(ps[:], wp_t[:],
                                         st[:, w0 + R:w0 + R + 512],
                                         start=True, stop=True)
                        nc.scalar.activation(oc[0:m, oi, w0:w0 + 512],
                                             ps[0:m, :], act_copy)
                    # One output DMA per pair of tiles, triggered from the
                    # scalar queue right after the ACT copies it needs.
                    if t == 8:
                        nc.scalar.dma_start(out[s, 8 * TILE_OUT:H, :],
                                            oc[0:64, 0, :])
                    elif t % 2 == 1:
                        nc.scalar.dma_start(dst_rows(s, t - 1, 2),
                                            oc[:, 0:2, :])

    nc.compile()
    return nc


def _get_nc():
    if "nc" not in _COMPILED:
        _COMPILED["nc"] = _build()
    return _COMPILED["nc"]


def _in_maps(x: np.ndarray):
    xf = np.ascontiguousarray(np.asarray(x, dtype=np.float32)).reshape(
        N_CORES * SLICES_PER_CORE, H, W)
    wp_np = _band_weights()
    return [{
        "x": xf[c * SLICES_PER_CORE:(c + 1) * SLICES_PER_CORE],
        "wp": wp_np,
    } for c in range(N_CORES)]


def kernel(x: np.ndarray) -> np.ndarray:
    from concourse.bass_utils import run_bass_kernel_spmd

    nc = _get_nc()
    res = run_bass_kernel_spmd(nc, _in_maps(x), core_ids=list(range(N_CORES)))
    outs = [res.results[c]["out"] for c in range(N_CORES)]
    return np.concatenate(outs, axis=0).reshape(8, 3, H, W)



# revision 2
# speedup vs baseline: 1.2474x; 1.2474x over previous
"""Trainium2 Bass kernel: separable box filter (radius 4) on (8,3,1024,1024) fp32.

Equivalent to the reference:
    box(x) = diff(cumsum(diff(cumsum(x, H), H), W), W)    # truncated 9x9 box sum

Strategy (pure data parallel over the 24 (n,c) slices, 3 per core):
  - All device I/O and SBUF compute in fp16 (host casts f32<->f16). The
    rel-err budget (2e-2) dwarfs fp16 quantization (~1e-3 worst case); the
    DVE scan carries its state in fp32 internally regardless of operand
    dtype, so there is no drift along the 1024-wide scan.
  - W pass entirely on DVE in ONE tensor_tensor_scan per tile:
        state[t] = state[t-1] + xpad[t] - xpad[t-9]
    over a zero-padded row buffer xpad = [0 x9 | x | 0 x4], which yields the
    truncated 9-tap running box sum S with S[w+4] = boxW(x)[w].
  - H pass on the PE: overlapping input tiles of 128 rows produce 120 output
    rows each via one banded weight matrix W[m, k] = 1 iff m <= k <= m+8
    (tile covers global rows 120t-4 .. 120t+123).  fp16 matmul = 1 cycle/row
    (fp32 needs 2 half-rate passes = 4 cycles/row).
  - ACT copies PSUM (f32) -> SBUF (fp16) in one [120,1024] instruction per
    tile (spans 2 PSUM banks), DMA out fp16.
"""

import numpy as np

H = 1024
W = 1024
R = 4
D = 2 * R + 1  # 9-tap window
N_CORES = 8
SLICES_PER_CORE = 3  # 8*3 = 24 (n,c) slices / 8 cores
TILE_OUT = 120  # output rows per PE tile (128 input rows - 2*R)
N_TILES = 9  # ceil(1024 / 120); last tile emits 64 rows
P_W = D + W + R  # 9 left zeros + 1024 data + 4 right zeros
S_W = W + R  # scan output length (box sums ending at 0..1027)

_COMPILED = {}


def _band_weights():
    """lhsT for the H-pass band matmul: [K=128, M=120], lhsT[k, m] = 1 iff
    m <= k <= m+8 (out row m consumes in rows m..m+8 of the tile)."""
    k = np.arange(128)[:, None]
    m = np.arange(TILE_OUT)[None, :]
    return ((m <= k) & (k <= m + 2 * R)).astype(np.float16)


def _build():
    from concourse import bacc, mybir
    from concourse.tile import TileContext

    f16 = mybir.dt.float16
    f32 = mybir.dt.float32
    nc = bacc.Bacc("TRN2", target_bir_lowering=False, debug=False,
                   num_devices=N_CORES)

    x = nc.dram_tensor("x", (SLICES_PER_CORE, H, W), f16,
                       kind="ExternalInput").ap()
    wp = nc.dram_tensor("wp", (128, TILE_OUT), f16, kind="ExternalInput").ap()
    out = nc.dram_tensor("out", (SLICES_PER_CORE, H, W), f16,
                         kind="ExternalOutput").ap()

    add = mybir.AluOpType.add
    sub = mybir.AluOpType.subtract
    act_copy = mybir.ActivationFunctionType.Copy

    from concourse.ap import AP

    xh = x.tensor
    oh = out.tensor

    def src_windows(s, t0, nt):
        # Overlapping 128-row windows: rows 120*t - 4 + p for t in
        # [t0, t0+nt), p in [0, 128).  Iteration order must match the
        # SBUF dest [p, t, w].
        off = s * H * W + (TILE_OUT * t0 - R) * W
        return AP(xh, off, [[W, 128], [TILE_OUT * W, nt], [1, W]])

    def dst_rows(s, t0, nt):
        # Output rows 120*t + p for t in [t0, t0+nt), p in [0, 120).
        off = s * H * W + TILE_OUT * t0 * W
        return AP(oh, off, [[W, TILE_OUT], [TILE_OUT * W, nt], [1, W]])

    with TileContext(nc) as tc:
        with tc.tile_pool(name="wts", bufs=1) as wpool, \
             tc.tile_pool(name="xp", bufs=1) as xpool, \
             tc.tile_pool(name="sc", bufs=8) as spool, \
             tc.tile_pool(name="outp", bufs=10) as opool, \
             tc.tile_pool(name="ps", bufs=4, space="PSUM") as pspool:
            wp_t = wpool.tile([128, TILE_OUT], f16)
            nc.sync.dma_start(wp_t[:], wp[:])

            # 9 persistent input buffers: chunk c uses buffer t (= c mod 9),
            # so buffer 0 always hosts t=0 tiles and buffer 8 always t=8.
            # All constant zero regions (row pads, out-of-image partition
            # ranges) are initialized ONCE here and never rewritten -- no
            # per-chunk memsets, no WAR edges in steady state.
            xbufs = []
            for t in range(N_TILES):
                xb = xpool.tile([128, P_W], f16, tag=f"xc{t}")
                nc.gpsimd.memset(xb[:, 0:D], 0.0)
                nc.gpsimd.memset(xb[:, D + W:P_W], 0.0)
                if t == 0:
                    nc.gpsimd.memset(xb[0:32, :], 0.0)
                if t == 8:
                    nc.gpsimd.memset(xb[64:128, :], 0.0)
                xbufs.append(xb)

            for s in range(SLICES_PER_CORE):
                for t in range(N_TILES):
                    xc = xbufs[t]
                    ineng = nc.sync
                    if t == 0:
                        ineng.dma_start(xc[4:128, D:D + W], x[s, 0:124, :])
                    elif t == 8:
                        ineng.dma_start(xc[0:68, D:D + W],
                                        x[s, 8 * TILE_OUT - R:H, :])
                    else:
                        ineng.dma_start(xc[:, D:D + W],
                                        src_windows(s, t, 1)[:, 0, :])

                    if t % 2 == 0:
                        oc = opool.tile([TILE_OUT, 2, W], f16, tag="oc")
                    oi = t % 2
                    m = min(TILE_OUT, H - TILE_OUT * t)  # output rows
                    # Running 9-tap box sum along W:
                    #   S[i] = S[i-1] + xpad[i] - xpad[i-9], i = 0..1027
                    # so S[w+4] = truncated boxW(x)[w].
                    st = spool.tile([128, S_W], f16)
                    nc.vector.tensor_tensor_scan(
                        st[:, :], xc[:, D:P_W], xc[:, 0:S_W], 0.0,
                        add, sub)
                    # One 2-bank PSUM tile per input tile; 2 matmuls fill the
                    # halves, one ACT instruction drains both.
                    ps = pspool.tile([TILE_OUT, 2 * 512], f32)
                    for hf in range(2):
                        w0 = 512 * hf
                        nc.tensor.matmul(ps[:, w0:w0 + 512], wp_t[:],
                                         st[:, w0 + R:w0 + R + 512],
                                         start=True, stop=True)
                    nc.scalar.activation(oc[0:m, oi, :], ps[0:m, :], act_copy)
                    # One output DMA per pair of tiles, triggered from the
                    # scalar queue right after the ACT copies it needs.
                    if t == 8:
                        nc.scalar.dma_start(out[s, 8 * TILE_OUT:H, :],
                                            oc[0:64, 0, :])
                    elif t % 2 == 1:
                        nc.scalar.dma_start(dst_rows(s, t - 1, 2),
                                            oc[:, 0:2, :])

    nc.compile()
    return nc


def _get_nc():
    if "nc" not in _COMPILED:
        _COMPILED["nc"] = _build()
    return _COMPILED["nc"]


def _in_maps(x: np.ndarray):
    xf = np.ascontiguousarray(np.asarray(x, dtype=np.float16)).reshape(
        N_CORES * SLICES_PER_CORE, H, W)
    wp_np = _band_weights()
    return [{
        "x": xf[c * SLICES_PER_CORE:(c + 1) * SLICES_PER_CORE],
        "wp": wp_np,
    } for c in range(N_CORES)]


def kernel(x: np.ndarray) -> np.ndarray:
    from concourse.bass_utils import run_bass_kernel_spmd

    nc = _get_nc()
    res = run_bass_kernel_spmd(nc, _in_maps(x), core_ids=list(range(N_CORES)))
    outs = [res.results[c]["out"] for c in range(N_CORES)]
    return np.concatenate(outs, axis=0).reshape(8, 3, H, W).astype(np.float32)
